# revision 15
# baseline (speedup 1.0000x reference)
"""Trainium2 Bass kernel for nn_DenseRED_SN (per-pixel spectral-norm dense reduce).

Math (full problem):
    w_mat = weight.reshape(H*W, C)
    sigma[p]  = ||w_mat[p, :]||_2                       (per-pixel L2 norm)
    out[b, 0, p] = (sum_c x[b, c, p] * w_mat[p, c]) / sigma[p] + bias[p]

Sharding: pixel-parallel over H across the 8 cores (32 image rows each).
Each core's slice of x / weight / bias is host-repacked (pure layout, no
arithmetic) into an SBUF-friendly "channel + pixel-half on partitions"
layout:
    partition p = h*64 + c   (h = pixel-half 0/1, c = channel)
    x_core[b, p, f]  = x[b, c, pix]  with pix = h*4096 + f
    w_core[p, f]     = w_mat[pix, c]
On-chip per core:
    sq    = w ⊙ w                                (VectorE)
    sig2  = ones_rep.T @ sq                      (PE; [32, F] replicated per batch-row)
    sigma = sqrt(sig2)                           (ScalarE, PSUM→SBUF)
    rsig  = 1/sigma                              (VectorE reciprocal_approx_fast)
    per batch b: prod = x_b ⊙ w                  (VectorE)
                 S_b  = ones2.T @ prod           (PE, float32r, N=512 chunks)
                 out_all[2b:2b+2] = copy(S_b)    (ScalarE PSUM→SBUF drain)
    out_all = out_all ⊙ rsig + bias              (VectorE tail)
"""

import os

import numpy as np

H, W, C, B = 256, 256, 64, 16
NCORES = 8
ROWS = H // NCORES        # 32 image rows per core
PIX = ROWS * W            # 8192 pixels per core
HALF = PIX // 2           # 4096 (free-dim size; two pixel halves on partitions)
NCHUNK = 512              # matmul moving free dim (one PSUM bank of fp32)
GRP = 2048                # psum tile free size (4 banks); 2 groups per batch

_cache = {}


def _ensure_jax_platform():
    # bass2jax executes through the axon PJRT backend; make sure a
    # JAX_PLATFORMS=cpu pin from a caller does not hide the neuron devices.
    plat = os.environ.get("JAX_PLATFORMS")
    if plat is not None and "axon" not in plat and "neuron" not in plat:
        del os.environ["JAX_PLATFORMS"]


def _build_nc(use_f32r=True):
    import concourse.bass as bass
    import concourse.tile as tile
    from concourse import bacc, mybir

    f32 = mybir.dt.float32
    f32r = mybir.dt.float32r

    # Bacc (not raw Bass): its compile() pass lowers multi-wait instructions
    # into event-semaphore/NOP form — the raw 64B ISA slots hold only one
    # sync wait, so a plain Bass build fails walrus codegen on any
    # double-buffered pipeline.
    nc = bacc.Bacc("TRN2", target_bir_lowering=False, debug=False)

    x_d = nc.dram_tensor("x", [B, 128, HALF], f32, kind="ExternalInput")
    w_d = nc.dram_tensor("w", [128, HALF], f32, kind="ExternalInput")
    bias_d = nc.dram_tensor("bias", [2, HALF], f32, kind="ExternalInput")
    oblk_d = nc.dram_tensor("ones_blk", [128, B * 32], f32, kind="ExternalInput")
    orep_d = nc.dram_tensor("ones_rep", [128, 32], f32, kind="ExternalInput")
    out_d = nc.dram_tensor("out", [2 * B, HALF], f32, kind="ExternalOutput")

    with tile.TileContext(nc) as tc:
        with (
            tc.tile_pool(name="const", bufs=1) as const_pool,
            tc.tile_pool(name="xin", bufs=3) as x_pool,
            tc.tile_pool(name="prod", bufs=2) as prod_pool,
            tc.tile_pool(name="ps", bufs=2, space="PSUM") as psum_pool,
        ):
            # ---- constants / weight ----
            w_sb = const_pool.tile([128, HALF], f32)
            nc.sync.dma_start(out=w_sb[:], in_=w_d[:, :])

            red_dt = f32r if use_f32r else f32

            # ones_blk[:, b, :] is batch b's reduction stationary: column
            # 2b sums partitions 0:64 (pixel half 0), column 2b+1 sums
            # partitions 64:128 (half 1).  All batches accumulate into one
            # [32, GRP] PSUM tile, so the drain is 2 ops instead of 32 and
            # every SBUF compute access stays 32-partition aligned.
            # ones_rep[k, m] = 1 where (m % 2) == k//64 : same reduction but
            # writes the result replicated into all 16 batch-row pairs.
            # Both patterns are host-provided constant tables.
            ones_blk = const_pool.tile([128, B, 32], red_dt)
            nc.sync.dma_start(
                out=ones_blk[:],
                in_=oblk_d[:, :].bitcast(red_dt) if use_f32r else oblk_d[:, :],
            )
            ones_rep = const_pool.tile([128, 32], f32)
            nc.sync.dma_start(out=ones_rep[:], in_=orep_d[:, :])

            # bias replicated to [32, HALF] via stride-0 DMA broadcast
            bias_rep = const_pool.tile([32, HALF], f32)
            b_ap = bias_d[:, :]
            bias_bcast = bass.AP(
                tensor=b_ap.tensor,
                offset=b_ap.offset,
                ap=[[0, 16]] + list(b_ap.ap),
            )
            nc.gpsimd.dma_start(out=bias_rep[:], in_=bias_bcast)

            # ---- main loop over batches ----
            out_all = const_pool.tile([32, HALF], f32)
            n_grp = HALF // GRP
            acc = [
                psum_pool.tile([32, GRP], f32, tag="ps", name=f"acc_{g}")
                for g in range(n_grp)
            ]
            for b in range(B):
                x_t = x_pool.tile([128, HALF], f32, tag="x")
                nc.gpsimd.dma_start(out=x_t[:], in_=x_d[b, :, :])

                prod = prod_pool.tile([128, HALF], red_dt, tag="prod")
                nc.vector.tensor_mul(prod[:], x_t[:], w_sb[:])

                for g in range(n_grp):
                    for j in range(GRP // NCHUNK):
                        nc.tensor.matmul(
                            acc[g][:, j * NCHUNK:(j + 1) * NCHUNK],
                            ones_blk[:, b, :],
                            prod[:, g * GRP + j * NCHUNK: g * GRP + (j + 1) * NCHUNK],
                            start=(b == 0),
                            stop=(b == B - 1),
                            skip_group_check=True,
                        )
            for g in range(n_grp):
                nc.scalar.copy(
                    out=out_all[:, g * GRP:(g + 1) * GRP],
                    in_=acc[g][:],
                )

            # ---- sigma (runs after the main loop; PSUM slots reused) ----
            sq = const_pool.tile([128, HALF], f32)
            nc.vector.tensor_mul(sq[:], w_sb[:], w_sb[:])

            sig_sb = const_pool.tile([32, HALF], f32)
            rsig = const_pool.tile([32, HALF], f32)
            for g in range(n_grp):
                ps_s = psum_pool.tile([32, GRP], f32, tag="ps", name=f"sig_ps_{g}")
                for j in range(GRP // NCHUNK):
                    nc.tensor.matmul(
                        ps_s[:, j * NCHUNK:(j + 1) * NCHUNK],
                        ones_rep[:],
                        sq[:, g * GRP + j * NCHUNK: g * GRP + (j + 1) * NCHUNK],
                        start=True,
                        stop=True,
                    )
                nc.scalar.activation(
                    out=sig_sb[:, g * GRP:(g + 1) * GRP],
                    in_=ps_s[:],
                    func=mybir.ActivationFunctionType.Sqrt,
                )
            nc.vector.reciprocal_approx_fast(out=rsig[:], in_=sig_sb[:])

            # ---- tail: scale by 1/sigma, add bias, store ----
            nc.vector.tensor_mul(out_all[:], out_all[:], rsig[:])
            nc.vector.tensor_add(out_all[:], out_all[:], bias_rep[:])
            nc.sync.dma_start(out=out_d[:, :], in_=out_all[:])

    nc.finalize()  # runs Bacc.compile(): reg alloc + multi-wait lowering
    return nc


def _ones_blk():
    if "ones_blk" not in _cache:
        o = np.zeros((128, B, 32), dtype=np.float32)
        p = np.arange(128)
        for b in range(B):
            o[p, b, 2 * b + (p // 64)] = 1.0
        _cache["ones_blk"] = np.ascontiguousarray(o.reshape(128, B * 32))
    return _cache["ones_blk"]


def _ones_rep():
    if "ones_rep" not in _cache:
        o = np.zeros((128, 32), dtype=np.float32)
        p = np.arange(128)[:, None]
        m = np.arange(32)[None, :]
        o[(m % 2) == (p // 64)] = 1.0
        _cache["ones_rep"] = np.ascontiguousarray(o)
    return _cache["ones_rep"]


def _shard_inputs(x, weight, bias):
    """Host-side (layout only) sharding/packing. Returns list of 8 input maps."""
    x = np.asarray(x, dtype=np.float32)
    weight = np.asarray(weight, dtype=np.float32)
    bias = np.asarray(bias, dtype=np.float32)
    w_mat = weight.reshape(H * W, C)
    bias_flat = bias.reshape(H * W)

    in_maps = []
    for k in range(NCORES):
        r0 = k * ROWS
        xs = x[:, :, r0:r0 + ROWS, :].reshape(B, C, PIX)
        # [B, C, 2, HALF] -> [B, 2, C, HALF] -> [B, 128, HALF]
        x_core = np.ascontiguousarray(
            xs.reshape(B, C, 2, HALF).transpose(0, 2, 1, 3)
        ).reshape(B, 128, HALF)

        ws = w_mat[r0 * W:(r0 + ROWS) * W, :]          # [PIX, C]
        # -> [2, HALF, C] -> [2, C, HALF] -> [128, HALF]
        w_core = np.ascontiguousarray(
            ws.reshape(2, HALF, C).transpose(0, 2, 1)
        ).reshape(128, HALF)

        bias_core = np.ascontiguousarray(
            bias_flat[r0 * W:(r0 + ROWS) * W].reshape(2, HALF)
        )
        in_maps.append({
            "x": x_core,
            "w": w_core,
            "bias": bias_core,
            "ones_blk": _ones_blk(),
            "ones_rep": _ones_rep(),
        })
    return in_maps


def _unshard_output(results):
    out = np.zeros((B, 1, H, W), dtype=np.float32)
    for k in range(NCORES):
        r = np.asarray(results[k]["out"], dtype=np.float32)   # [32, HALF]
        out[:, 0, k * ROWS:(k + 1) * ROWS, :] = r.reshape(B, PIX).reshape(B, ROWS, W)
    return out


def _run(inputs, trace=False, use_f32r=True):
    _ensure_jax_platform()
    from concourse.bass_utils import run_bass_kernel_spmd

    key = ("nc", use_f32r)
    if key not in _cache:
        _cache[key] = _build_nc(use_f32r=use_f32r)
    nc = _cache[key]

    in_maps = _shard_inputs(inputs["x"], inputs["weight"], inputs["bias"])
    res = run_bass_kernel_spmd(
        nc, in_maps, core_ids=list(range(NCORES)), trace=trace
    )
    return _unshard_output(res.results), res


def kernel(x, weight, bias):
    out, _ = _run({"x": x, "weight": weight, "bias": bias})
    return out


# revision 16
# speedup vs baseline: 1.2724x; 1.2724x over previous
"""Trainium2 Bass kernel for nn_DenseRED_SN (per-pixel spectral-norm dense reduce).

Math (full problem):
    w_mat = weight.reshape(H*W, C)
    sigma[p]  = ||w_mat[p, :]||_2                       (per-pixel L2 norm)
    out[b, 0, p] = (sum_c x[b, c, p] * w_mat[p, c]) / sigma[p] + bias[p]

Sharding: pixel-parallel over H across the 8 cores (32 image rows each).
Each core's slice of x / weight / bias is host-repacked (pure layout, no
arithmetic) into an SBUF-friendly "channel + pixel-half on partitions"
layout:
    partition p = h*64 + c   (h = pixel-half 0/1, c = channel)
    x_core[b, p, f]  = x[b, c, pix]  with pix = h*4096 + f
    w_core[p, f]     = w_mat[pix, c]

On-chip per core (all arithmetic on device):
    sq    = w ⊙ w                                  (VectorE)
    sig   = sqrt(ones_rep.T @ sq)                  (PE + ScalarE, replicated
                                                    into all 16 batch-row pairs)
    rsig  = 1/sig                                  (VectorE reciprocal_approx_fast)
    bias2 = bias ⊙ sig                             (VectorE; bias pre-scaled so it
                                                    can ride the PSUM accumulation)
    per batch b: prod = x_b ⊙ w                    (VectorE, fp32r rounded)
                 acc[32, F] += ones_blk_b.T @ prod (PE, fp32r, one accumulating
                                                    PSUM tile per 2048-column group)
    acc += ones_bias.T @ bias2                     (PE, K=2 rank update)
    out_all = acc ⊙ rsig                           (VectorE, PSUM→SBUF drain+scale)
"""

import os

import numpy as np

H, W, C, B = 256, 256, 64, 16
NCORES = 8
ROWS = H // NCORES        # 32 image rows per core
PIX = ROWS * W            # 8192 pixels per core
HALF = PIX // 2           # 4096 (free-dim size; two pixel halves on partitions)
NCHUNK = 512              # matmul moving free dim (one PSUM bank of fp32)
GRP = 2048                # psum tile free size (4 banks); 2 groups per batch
X_BUFS = 4

_cache = {}


def _ensure_jax_platform():
    # bass2jax executes through the axon PJRT backend; make sure a
    # JAX_PLATFORMS=cpu pin from a caller does not hide the neuron devices.
    plat = os.environ.get("JAX_PLATFORMS")
    if plat is not None and "axon" not in plat and "neuron" not in plat:
        del os.environ["JAX_PLATFORMS"]


def _build_nc(use_f32r=True):
    import concourse.bass as bass
    import concourse.tile as tile
    from concourse import bacc, mybir

    f32 = mybir.dt.float32
    f32r = mybir.dt.float32r

    # Bacc (not raw Bass): its compile() pass lowers multi-wait instructions
    # into event-semaphore/NOP form — the raw 64B ISA slots hold only one
    # sync wait, so a plain Bass build fails walrus codegen on any
    # double-buffered pipeline.
    nc = bacc.Bacc("TRN2", target_bir_lowering=False, debug=False)

    x_d = nc.dram_tensor("x", [B, 128, HALF], f32, kind="ExternalInput")
    w_d = nc.dram_tensor("w", [128, HALF], f32, kind="ExternalInput")
    bias_d = nc.dram_tensor("bias", [2, HALF], f32, kind="ExternalInput")
    oblk_d = nc.dram_tensor("ones_blk", [128, B * 32], f32, kind="ExternalInput")
    orep_d = nc.dram_tensor("ones_rep", [128, 32], f32, kind="ExternalInput")
    obias_d = nc.dram_tensor("ones_bias", [2, 32], f32, kind="ExternalInput")
    out_d = nc.dram_tensor("out", [2 * B, HALF], f32, kind="ExternalOutput")

    n_grp = HALF // GRP

    with tile.TileContext(nc) as tc:
        with (
            tc.tile_pool(name="const", bufs=1) as const_pool,
            tc.tile_pool(name="xin", bufs=X_BUFS) as x_pool,
            tc.tile_pool(name="prod", bufs=2) as prod_pool,
            tc.tile_pool(name="ps", bufs=2, space="PSUM") as psum_pool,
        ):
            red_dt = f32r if use_f32r else f32

            def maybe_r(ap):
                return ap.bitcast(f32r) if use_f32r else ap

            # ---- constants / weight ----
            w_sb = const_pool.tile([128, HALF], f32)
            nc.sync.dma_start(out=w_sb[:], in_=w_d[:, :])

            ones_blk = const_pool.tile([128, B, 32], red_dt)
            nc.sync.dma_start(out=ones_blk[:], in_=maybe_r(oblk_d[:, :]))
            ones_rep = const_pool.tile([128, 32], f32)
            nc.sync.dma_start(out=ones_rep[:], in_=orep_d[:, :])
            ones_bias = const_pool.tile([2, 32], red_dt)
            nc.sync.dma_start(out=ones_bias[:], in_=maybe_r(obias_d[:, :]))
            bias_sb = const_pool.tile([2, HALF], f32)
            nc.sync.dma_start(out=bias_sb[:], in_=bias_d[:, :])

            # ---- sigma chain (overlaps the first batch DMAs) ----
            sq = prod_pool.tile([128, HALF], f32, tag="prod", name="sq")
            nc.vector.tensor_mul(sq[:], w_sb[:], w_sb[:])

            rsig = const_pool.tile([32, HALF], f32)
            for g in range(n_grp):
                ps_s = psum_pool.tile([32, GRP], f32, tag="ps", name=f"sig_ps_{g}")
                for j in range(GRP // NCHUNK):
                    nc.tensor.matmul(
                        ps_s[:, j * NCHUNK:(j + 1) * NCHUNK],
                        ones_rep[:],
                        sq[:, g * GRP + j * NCHUNK: g * GRP + (j + 1) * NCHUNK],
                        start=True,
                        stop=True,
                    )
                # rsig temporarily holds sigma; inverted in place below
                nc.scalar.activation(
                    out=rsig[:, g * GRP:(g + 1) * GRP],
                    in_=ps_s[:],
                    func=mybir.ActivationFunctionType.Sqrt,
                )
            # bias2 = bias * sigma (rows 0/1 of rsig still hold sigma here)
            bias2 = const_pool.tile([2, HALF], red_dt)
            nc.vector.tensor_mul(bias2[:], bias_sb[:], rsig[0:2, :])
            nc.vector.reciprocal_approx_fast(out=rsig[:], in_=rsig[:])

            # ---- main loop over batches: accumulate into PSUM ----
            acc = [
                psum_pool.tile([32, GRP], f32, tag="ps", name=f"acc_{g}")
                for g in range(n_grp)
            ]
            for b in range(B):
                x_t = x_pool.tile([128, HALF], f32, tag="x", name=f"x_{b}")
                nc.sync.dma_start(out=x_t[:], in_=x_d[b, :, :])

                prod = prod_pool.tile([128, HALF], red_dt, tag="prod",
                                      name=f"prod_{b}")
                nc.vector.tensor_mul(prod[:], x_t[:], w_sb[:])

                for g in range(n_grp):
                    for j in range(GRP // NCHUNK):
                        nc.tensor.matmul(
                            acc[g][:, j * NCHUNK:(j + 1) * NCHUNK],
                            ones_blk[:, b, :],
                            prod[:, g * GRP + j * NCHUNK: g * GRP + (j + 1) * NCHUNK],
                            start=(b == 0),
                            stop=False,
                            skip_group_check=True,
                        )
            # bias rank-2 update closes each accumulation group
            for g in range(n_grp):
                for j in range(GRP // NCHUNK):
                    nc.tensor.matmul(
                        acc[g][:, j * NCHUNK:(j + 1) * NCHUNK],
                        ones_bias[:],
                        bias2[:, g * GRP + j * NCHUNK: g * GRP + (j + 1) * NCHUNK],
                        start=False,
                        stop=True,
                        skip_group_check=True,
                    )

            # ---- tail: drain+scale by 1/sigma, store ----
            out_all = const_pool.tile([32, HALF], f32)
            for g in range(n_grp):
                nc.vector.tensor_mul(
                    out_all[:, g * GRP:(g + 1) * GRP],
                    acc[g][:],
                    rsig[:, g * GRP:(g + 1) * GRP],
                )
            nc.sync.dma_start(out=out_d[:, :], in_=out_all[:])

    nc.finalize()  # runs Bacc.compile(): reg alloc + multi-wait lowering
    return nc


def _ones_blk():
    if "ones_blk" not in _cache:
        o = np.zeros((128, B, 32), dtype=np.float32)
        p = np.arange(128)
        for b in range(B):
            o[p, b, 2 * b + (p // 64)] = 1.0
        _cache["ones_blk"] = np.ascontiguousarray(o.reshape(128, B * 32))
    return _cache["ones_blk"]


def _ones_rep():
    if "ones_rep" not in _cache:
        o = np.zeros((128, 32), dtype=np.float32)
        p = np.arange(128)[:, None]
        m = np.arange(32)[None, :]
        o[(m % 2) == (p // 64)] = 1.0
        _cache["ones_rep"] = np.ascontiguousarray(o)
    return _cache["ones_rep"]


def _ones_bias():
    if "ones_bias" not in _cache:
        o = np.zeros((2, 32), dtype=np.float32)
        h = np.arange(2)[:, None]
        m = np.arange(32)[None, :]
        o[(m % 2) == h] = 1.0
        _cache["ones_bias"] = np.ascontiguousarray(o)
    return _cache["ones_bias"]


def _shard_inputs(x, weight, bias):
    """Host-side (layout only) sharding/packing. Returns list of 8 input maps."""
    x = np.asarray(x, dtype=np.float32)
    weight = np.asarray(weight, dtype=np.float32)
    bias = np.asarray(bias, dtype=np.float32)
    w_mat = weight.reshape(H * W, C)
    bias_flat = bias.reshape(H * W)

    in_maps = []
    for k in range(NCORES):
        r0 = k * ROWS
        xs = x[:, :, r0:r0 + ROWS, :].reshape(B, C, PIX)
        # [B, C, 2, HALF] -> [B, 2, C, HALF] -> [B, 128, HALF]
        x_core = np.ascontiguousarray(
            xs.reshape(B, C, 2, HALF).transpose(0, 2, 1, 3)
        ).reshape(B, 128, HALF)

        ws = w_mat[r0 * W:(r0 + ROWS) * W, :]          # [PIX, C]
        # -> [2, HALF, C] -> [2, C, HALF] -> [128, HALF]
        w_core = np.ascontiguousarray(
            ws.reshape(2, HALF, C).transpose(0, 2, 1)
        ).reshape(128, HALF)

        bias_core = np.ascontiguousarray(
            bias_flat[r0 * W:(r0 + ROWS) * W].reshape(2, HALF)
        )
        in_maps.append({
            "x": x_core,
            "w": w_core,
            "bias": bias_core,
            "ones_blk": _ones_blk(),
            "ones_rep": _ones_rep(),
            "ones_bias": _ones_bias(),
        })
    return in_maps


def _unshard_output(results):
    out = np.zeros((B, 1, H, W), dtype=np.float32)
    for k in range(NCORES):
        r = np.asarray(results[k]["out"], dtype=np.float32)   # [32, HALF]
        out[:, 0, k * ROWS:(k + 1) * ROWS, :] = r.reshape(B, PIX).reshape(B, ROWS, W)
    return out


def _run(inputs, trace=False, use_f32r=True):
    _ensure_jax_platform()
    from concourse.bass_utils import run_bass_kernel_spmd

    key = ("nc", use_f32r)
    if key not in _cache:
        _cache[key] = _build_nc(use_f32r=use_f32r)
    nc = _cache[key]

    in_maps = _shard_inputs(inputs["x"], inputs["weight"], inputs["bias"])
    res = run_bass_kernel_spmd(
        nc, in_maps, core_ids=list(range(NCORES)), trace=trace
    )
    return _unshard_output(res.results), res


def kernel(x, weight, bias):
    out, _ = _run({"x": x, "weight": weight, "bias": bias})
    return out
